# revision 17
# baseline (speedup 1.0000x reference)
"""BlockXDiag (tri-diagonal block matrix × batch, periodic corners) on 8
Trainium2 NeuronCores.

Math (per reference): out_i = x_{i-1} @ A_i.T + x_i @ Wd_i.T + x_{i+1} @ Wu_i.T
for block-rows i in [0, 64), block size P=256, batch B=4096, with periodic
corner terms (x_63 @ Wtr.T into out_0, x_0 @ Wbl.T into out_63).

Sharding: output block-rows are split 8-per-core (weights split across
cores, which keeps per-core weight traffic at 6.3 MB and lets each core
stream only its 10-block x halo). Inputs are staged host-side as x^T so the
contraction dim lands on SBUF partitions; output is produced transposed and
un-transposed on the host.

Device kernel per core: out.T[q, b] tiles [128, 512] accumulated in PSUM
over 6 matmuls (3 source blocks x 2 k-halves), weights stationary.
"""
import numpy as np
import ml_dtypes

import concourse.bass as bass
import concourse.mybir as mybir
from concourse.tile import TileContext
from concourse.vector_clock import ScopedClock
from concourse.bass_utils import run_bass_kernel_spmd

M, P, B = 64, 256, 4096
NCORES = 8
BPC = M // NCORES          # output blocks per core: 8
NHALO = BPC + 2            # x blocks needed per core: 10
ROWS = NHALO * P           # x^T rows per core: 2560
BT = 512                   # batch-tile (matmul moving free dim)
NBT = B // BT              # 8

MODE = "bf16"              # "f32" | "f32r" | "bf16"
TRACE = False              # set by test harness to profile
REPEATS = 1                # extra timed executions (test harness only)
LAST_EXEC_NS = None
ALL_EXEC_NS = None

_DT = {
    "f32": (mybir.dt.float32, np.float32),
    "f32r": (mybir.dt.float32r, np.float32),
    "bf16": (mybir.dt.bfloat16, ml_dtypes.bfloat16),
}


# ---------------------------------------------------------------------------
# Workarounds for the pinned walrus build's 1-wait-per-instruction cap.
# Tile's tail drain stuffs every outstanding sem wait onto one Drain, and
# self-loading fp32/fp32r matmuls can carry >1 wait with no Ldweights to
# spill to. Split both across extra same-engine instructions.
def _patched_drain_and_barrier(self, tick_clock, wait_clock):
    drain_inst = self.nc.sync.drain()
    wait_clock.add_sem_waits(
        drain_inst.ins, ScopedClock({None: tick_clock.global_clock})
    )
    si = drain_inst.ins.sync_info
    waits = list(si.on_wait)
    if len(waits) > 1:
        drain_inst.ins.sync_info = mybir.SyncInfo(
            on_wait=[waits[0]], on_update=list(si.on_update)
        )
        for w in waits[1:]:
            d2 = self.nc.sync.drain()
            d2.ins.sync_info = mybir.SyncInfo(on_wait=[w], on_update=[])
    self.nc.all_engine_barrier()
    assert self.sems is not None
    popped = self.nc._tile_sem_poison_stack.pop()
    assert popped is self._sem_poison
    self.nc.clear_and_free_semaphores(list(self.sems.allocated().values()))
    self.nc.all_engine_barrier()


def _apply_tile_patch():
    TileContext._drain_and_barrier = _patched_drain_and_barrier


def _install_profile_shim():
    """Make trace=True work in this container: provide the missing
    antenv.axon_hooks module (NTFF capture via ctypes into libaxon_pjrt.so)
    and skip the bucket upload of artifacts."""
    import sys, types, ctypes, contextlib
    import concourse.bass_utils as bu
    bu.upload_artifacts = lambda tmpdir: tmpdir
    try:
        from antenv.axon_hooks import get_axon_ntff_profile_hook  # noqa
        return
    except ImportError:
        pass
    so_path = "/opt/axon/libaxon_pjrt.so"
    lib = ctypes.CDLL(so_path)
    if not hasattr(lib, "axon_start_nrt_profile"):
        return
    lib.axon_start_nrt_profile.argtypes = [
        ctypes.POINTER(ctypes.c_int64), ctypes.c_size_t]
    lib.axon_start_nrt_profile.restype = ctypes.c_int64
    lib.axon_stop_nrt_profile.argtypes = [ctypes.c_char_p]
    lib.axon_stop_nrt_profile.restype = ctypes.c_int64

    @contextlib.contextmanager
    def _hook(output_dir, device_ids):
        import jax
        jax.devices()
        if device_ids:
            ids = (ctypes.c_int64 * len(device_ids))(*device_ids)
            rc = lib.axon_start_nrt_profile(ids, len(device_ids))
        else:
            rc = lib.axon_start_nrt_profile(None, 0)
        if rc != 0:
            raise RuntimeError(f"axon_start_nrt_profile rc={rc}")
        try:
            yield
        finally:
            n = lib.axon_stop_nrt_profile(str(output_dir).encode())
            print(f"profile: {n} file(s) written to {output_dir}")

    mod = types.ModuleType("antenv.axon_hooks")
    mod.get_axon_ntff_profile_hook = lambda: _hook
    mod.set_axon_ntff_profile_hook = lambda h: None
    sys.modules["antenv.axon_hooks"] = mod
    import antenv
    antenv.axon_hooks = mod


def _hoist_excess_waits(nc):
    """Any non-EventSemaphore instruction may carry at most 1 sem wait on
    this walrus build; move extras onto inserted same-engine NoOps."""
    for fn in nc.m.functions:
        for bb in fn.blocks:
            insts = bb.instructions
            newlist = []
            changed = False
            for inst in insts:
                si = inst.sync_info
                cap = 2 if isinstance(inst, mybir.InstEventSemaphore) else 1
                if si is not None and len(si.on_wait) > cap:
                    waits = list(si.on_wait)
                    for i, w in enumerate(waits[cap:]):
                        newlist.append(mybir.InstNoOp(
                            name=f"{inst.name}_waitnop{i}",
                            engine=inst.engine,
                            bass_nofuse=True,
                            sync_info=mybir.SyncInfo(on_wait=[w], on_update=[]),
                        ))
                    inst.sync_info = mybir.SyncInfo(
                        on_wait=waits[:cap], on_update=list(si.on_update))
                    changed = True
                newlist.append(inst)
            if changed:
                insts.clear()
                insts.extend(newlist)


# ---------------------------------------------------------------------------
# All DRAM tensors are staged host-side in partition-major, tile-contiguous
# layouts so every dma_start is a handful of large per-partition runs (fast
# HWDGE descriptor generation, full line rate):
#   x_d [128, NBT, 20, BT]   (p, batch-tile, block-k-half, batch)
#   w_d [128, 48, 256]       (p=k-half, j=(li,s,kh), q)
#   o_d [128, NBT, 8, 2, BT] (p, batch-tile, li, qh, batch)
# Startup interleaves per-li weight chunks with per-block x chunks on the
# single HW DMA queue so the first matmul only waits for ~1 MB, and each
# x tile is prefetched at the top of the previous tile's compute so it is
# not queued behind that tile's output stores.
def _build_nc(mode):
    dt_in, _ = _DT[mode]
    f32 = mybir.dt.float32
    # bf16 mode also writes the output in bf16 (host upcasts): halves the
    # store traffic, well inside the 2e-2 gate.
    dt_out = dt_in if mode == "bf16" else f32
    nc = bass.Bass()
    x_d = nc.dram_tensor("x", [128, NBT, NHALO * 2, BT], dt_in,
                         kind="ExternalInput")
    w_d = nc.dram_tensor("w", [128, BPC * 3 * 2, P], dt_in,
                         kind="ExternalInput")
    o_d = nc.dram_tensor("o", [128, NBT, BPC, 2, BT], dt_out,
                         kind="ExternalOutput")

    with TileContext(nc) as tc:
        with tc.tile_pool(name="wpool", bufs=1) as wpool, \
             tc.tile_pool(name="xpool", bufs=2) as xpool, \
             tc.tile_pool(name="opool", bufs=4) as opool, \
             tc.tile_pool(name="pspool", bufs=6, space="PSUM") as pspool, \
             tc.tile_pool(name="ps2pool", bufs=2, space="PSUM") as ps2pool:
            w_sb = wpool.tile([128, BPC * 3 * 2, P], dt_in)
            xt0 = xpool.tile([128, NHALO * 2, BT], dt_in, tag="x")

            # PE warm-up: the HAM clock gate holds the PE at 1.2 GHz until
            # ~3.4 us of sustained activity. Run dummy matmuls on zeroed
            # scratch during the startup-DMA dead time so the real stream
            # begins (nearly) at 2.4 GHz. Sized to finish just before the
            # first x block lands (~10 us).
            warm_w = wpool.tile([128, 128], dt_in, tag="warmw")
            nc.vector.memset(warm_w, 0.0)
            ps_warm = pspool.tile([128, BT], f32, tag="ps")
            for _ in range(12):
                nc.tensor.matmul(ps_warm[:, 0:128], warm_w, warm_w,
                                 start=True, stop=True)

            def w_chunk(j0, j1):
                nc.sync.dma_start(out=w_sb[:, j0:j1, :], in_=w_d[:, j0:j1, :])

            def x0_chunk(blk):
                nc.sync.dma_start(out=xt0[:, blk * 2:blk * 2 + 2, :],
                                  in_=x_d[:, 0, blk * 2:blk * 2 + 2, :])

            # startup drip, sized so HWDGE descriptor generation (~0.5 us
            # per dma_start on the sync engine) stays ahead of the queue:
            # the first group needs w[0:6] + x blocks 0-2, each li after
            # consumes one x block / 2.56 us of compute.
            w_chunk(0, 2)
            x0_chunk(0)
            w_chunk(2, 6)
            x0_chunk(1)
            x0_chunk(2)
            for li in range(1, BPC):
                if li % 2 == 1:
                    w_chunk(li * 6, min((li + 2) * 6, BPC * 6))
                x0_chunk(li + 2)

            xt = xt0
            for bt in range(NBT):
                # stores ride the scalar engine's own HW queue
                # (qActDynamicHW) so they never head-of-line block x loads
                # on the sync queue, and vice versa.
                last_tile = bt == NBT - 1
                if not last_tile:
                    xt_next = xpool.tile([128, NHALO * 2, BT], dt_in, tag="x")
                    nc.sync.dma_start(out=xt_next, in_=x_d[:, bt + 1])
                for g in range(BPC // 2):   # output pairs of block rows
                    for lj in range(2):
                        li = g * 2 + lj
                        if last_tile or lj == 0:
                            ot = opool.tile(
                                [128, 2, 2, BT], dt_out, tag="o")
                        for qh in range(2):
                            final = last_tile and li == BPC - 1 and qh == 1
                            # the very last group runs as two 256-column
                            # half-groups so the cast+store chain after the
                            # final matmul is half as long
                            bhs = (0, 1) if final else (0,)
                            for bh in bhs:
                                bsl = (slice(bh * 256, bh * 256 + 256)
                                       if final else slice(None))
                                if final:
                                    ps = ps2pool.tile([128, 256], f32,
                                                      tag="p2", name="ps2")
                                else:
                                    ps = pspool.tile([128, BT], f32,
                                                     tag="ps", name="ps")
                                for s in range(3):
                                    for kh in range(2):
                                        nc.tensor.matmul(
                                            ps,
                                            w_sb[:, (li * 3 + s) * 2 + kh,
                                                 qh * 128:(qh + 1) * 128],
                                            xt[:, (li + s) * 2 + kh, bsl],
                                            start=(s == 0 and kh == 0),
                                            stop=(s == 2 and kh == 1),
                                        )
                                nc.vector.tensor_copy(
                                    out=ot[:, 0 if last_tile else lj, qh,
                                           bsl],
                                    in_=ps)
                                if final:
                                    nc.scalar.dma_start(
                                        out=o_d[:, bt, li:li + 1, qh:qh + 1,
                                                bsl],
                                        in_=ot[:, 0:1, qh:qh + 1, bsl])
                            if last_tile and li == BPC - 1 and qh == 0:
                                nc.scalar.dma_start(
                                    out=o_d[:, bt, li:li + 1, 0:1, :],
                                    in_=ot[:, 0:1, 0:1, :])
                        if last_tile and li < BPC - 1:
                            # per-block stores drain the store queue before
                            # the final matmul, shrinking the exit tail
                            nc.scalar.dma_start(
                                out=o_d[:, bt, li:li + 1, :, :],
                                in_=ot[:, 0:1, :, :])
                    if not last_tile:
                        nc.scalar.dma_start(
                            out=o_d[:, bt, g * 2:g * 2 + 2, :, :], in_=ot)
                if not last_tile:
                    xt = xt_next
    _hoist_excess_waits(nc)
    return nc


def _host_prep(x, Wd, Wu, Wl, Wtr, Wbl, np_dt):
    x = np.asarray(x, np.float32)
    Wd, Wu, Wl = np.asarray(Wd, np.float32), np.asarray(Wu, np.float32), np.asarray(Wl, np.float32)
    Wtr, Wbl = np.asarray(Wtr, np.float32), np.asarray(Wbl, np.float32)

    xT = np.ascontiguousarray(x.T).astype(np_dt)         # [M*P, B]
    A = np.concatenate([Wtr[None], Wl], axis=0)          # weight applied to x_{i-1}
    Bst = Wd                                             # weight applied to x_i
    C = np.concatenate([Wu, Wbl[None]], axis=0)          # weight applied to x_{i+1}
    WT = np.stack([A, Bst, C], axis=1)                   # [64, 3, q, p]
    WT = np.ascontiguousarray(WT.transpose(0, 1, 3, 2))  # [64, 3, p, q]

    in_maps = []
    for c in range(NCORES):
        lo = (8 * c - 1) * P
        hi = (8 * c + 9) * P
        if lo < 0:
            xc = np.concatenate([xT[lo:], xT[:hi]], axis=0)
        elif hi > M * P:
            xc = np.concatenate([xT[lo:], xT[:hi - M * P]], axis=0)
        else:
            xc = xT[lo:hi]                               # [2560, 4096]
        # [2560, B] -> [128p, NBT, 20t, BT] partition-major tiled
        xc = np.ascontiguousarray(
            xc.reshape(NHALO * 2, 128, NBT, BT).transpose(1, 2, 0, 3))
        wc = WT[8 * c:8 * c + 8].reshape(BPC * 3 * 2, 128, P).astype(np_dt)
        wc = np.ascontiguousarray(wc.transpose(1, 0, 2))  # [128, 48, 256]
        in_maps.append({"x": xc, "w": wc})
    return in_maps


def kernel(x, Wd, Wu, Wl, Wtr, Wbl):
    global LAST_EXEC_NS
    _apply_tile_patch()
    if TRACE:
        try:
            _install_profile_shim()
        except Exception as e:
            print(f"profile shim failed ({e}); running without trace")
    dt_in, np_dt = _DT[MODE]
    nc = _build_nc(MODE)
    in_maps = _host_prep(x, Wd, Wu, Wl, Wtr, Wbl, np_dt)
    res = run_bass_kernel_spmd(
        nc, in_maps, core_ids=list(range(NCORES)), trace=TRACE)
    LAST_EXEC_NS = res.exec_time_ns
    if TRACE and REPEATS > 1:
        global ALL_EXEC_NS
        ALL_EXEC_NS = [res.exec_time_ns]
        for _ in range(REPEATS - 1):
            r2 = run_bass_kernel_spmd(
                nc, in_maps, core_ids=list(range(NCORES)), trace=True)
            ALL_EXEC_NS.append(r2.exec_time_ns)
        LAST_EXEC_NS = min(t for t in ALL_EXEC_NS if t)
    out = np.empty((B, M, P), dtype=np.float32)
    for c in range(NCORES):
        oc = res.results[c]["o"]                  # [128p, NBT, 8li, 2qh, BT]
        # -> [bt, b, li, qh, p] -> [B, 8, 256]
        out[:, 8 * c:8 * c + 8, :] = (
            oc.transpose(1, 4, 2, 3, 0).reshape(B, BPC, P))
    return np.ascontiguousarray(out.reshape(B, M * P))   # [B, M*P] float32



# revision 19
# speedup vs baseline: 1.0009x; 1.0009x over previous
"""BlockXDiag (tri-diagonal block matrix × batch, periodic corners) on 8
Trainium2 NeuronCores.

Math (per reference): out_i = x_{i-1} @ A_i.T + x_i @ Wd_i.T + x_{i+1} @ Wu_i.T
for block-rows i in [0, 64), block size P=256, batch B=4096, with periodic
corner terms (x_63 @ Wtr.T into out_0, x_0 @ Wbl.T into out_63).

Sharding: output block-rows are split 8-per-core (weights split across
cores, which keeps per-core weight traffic at 6.3 MB and lets each core
stream only its 10-block x halo). Inputs are staged host-side as x^T so the
contraction dim lands on SBUF partitions; output is produced transposed and
un-transposed on the host.

Device kernel per core: out.T[q, b] tiles [128, 512] accumulated in PSUM
over 6 matmuls (3 source blocks x 2 k-halves), weights stationary.
"""
import numpy as np
import ml_dtypes

import concourse.bass as bass
import concourse.mybir as mybir
from concourse.tile import TileContext
from concourse.vector_clock import ScopedClock
from concourse.bass_utils import run_bass_kernel_spmd

M, P, B = 64, 256, 4096
NCORES = 8
BPC = M // NCORES          # output blocks per core: 8
NHALO = BPC + 2            # x blocks needed per core: 10
ROWS = NHALO * P           # x^T rows per core: 2560
BT = 512                   # batch-tile (matmul moving free dim)
NBT = B // BT              # 8

MODE = "bf16"              # "f32" | "f32r" | "bf16"
TRACE = False              # set by test harness to profile
REPEATS = 1                # extra timed executions (test harness only)
LAST_EXEC_NS = None
ALL_EXEC_NS = None

_DT = {
    "f32": (mybir.dt.float32, np.float32),
    "f32r": (mybir.dt.float32r, np.float32),
    "bf16": (mybir.dt.bfloat16, ml_dtypes.bfloat16),
}


# ---------------------------------------------------------------------------
# Workarounds for the pinned walrus build's 1-wait-per-instruction cap.
# Tile's tail drain stuffs every outstanding sem wait onto one Drain, and
# self-loading fp32/fp32r matmuls can carry >1 wait with no Ldweights to
# spill to. Split both across extra same-engine instructions.
def _patched_drain_and_barrier(self, tick_clock, wait_clock):
    drain_inst = self.nc.sync.drain()
    wait_clock.add_sem_waits(
        drain_inst.ins, ScopedClock({None: tick_clock.global_clock})
    )
    si = drain_inst.ins.sync_info
    waits = list(si.on_wait)
    if len(waits) > 1:
        drain_inst.ins.sync_info = mybir.SyncInfo(
            on_wait=[waits[0]], on_update=list(si.on_update)
        )
        for w in waits[1:]:
            d2 = self.nc.sync.drain()
            d2.ins.sync_info = mybir.SyncInfo(on_wait=[w], on_update=[])
    self.nc.all_engine_barrier()
    assert self.sems is not None
    popped = self.nc._tile_sem_poison_stack.pop()
    assert popped is self._sem_poison
    self.nc.clear_and_free_semaphores(list(self.sems.allocated().values()))
    self.nc.all_engine_barrier()


def _apply_tile_patch():
    TileContext._drain_and_barrier = _patched_drain_and_barrier


def _install_profile_shim():
    """Make trace=True work in this container: provide the missing
    antenv.axon_hooks module (NTFF capture via ctypes into libaxon_pjrt.so)
    and skip the bucket upload of artifacts."""
    import sys, types, ctypes, contextlib
    import concourse.bass_utils as bu
    bu.upload_artifacts = lambda tmpdir: tmpdir
    try:
        from antenv.axon_hooks import get_axon_ntff_profile_hook  # noqa
        return
    except ImportError:
        pass
    so_path = "/opt/axon/libaxon_pjrt.so"
    lib = ctypes.CDLL(so_path)
    if not hasattr(lib, "axon_start_nrt_profile"):
        return
    lib.axon_start_nrt_profile.argtypes = [
        ctypes.POINTER(ctypes.c_int64), ctypes.c_size_t]
    lib.axon_start_nrt_profile.restype = ctypes.c_int64
    lib.axon_stop_nrt_profile.argtypes = [ctypes.c_char_p]
    lib.axon_stop_nrt_profile.restype = ctypes.c_int64

    @contextlib.contextmanager
    def _hook(output_dir, device_ids):
        import jax
        jax.devices()
        if device_ids:
            ids = (ctypes.c_int64 * len(device_ids))(*device_ids)
            rc = lib.axon_start_nrt_profile(ids, len(device_ids))
        else:
            rc = lib.axon_start_nrt_profile(None, 0)
        if rc != 0:
            raise RuntimeError(f"axon_start_nrt_profile rc={rc}")
        try:
            yield
        finally:
            n = lib.axon_stop_nrt_profile(str(output_dir).encode())
            print(f"profile: {n} file(s) written to {output_dir}")

    mod = types.ModuleType("antenv.axon_hooks")
    mod.get_axon_ntff_profile_hook = lambda: _hook
    mod.set_axon_ntff_profile_hook = lambda h: None
    sys.modules["antenv.axon_hooks"] = mod
    import antenv
    antenv.axon_hooks = mod


def _hoist_excess_waits(nc):
    """Any non-EventSemaphore instruction may carry at most 1 sem wait on
    this walrus build; move extras onto inserted same-engine NoOps."""
    for fn in nc.m.functions:
        for bb in fn.blocks:
            insts = bb.instructions
            newlist = []
            changed = False
            for inst in insts:
                si = inst.sync_info
                cap = 2 if isinstance(inst, mybir.InstEventSemaphore) else 1
                if si is not None and len(si.on_wait) > cap:
                    waits = list(si.on_wait)
                    for i, w in enumerate(waits[cap:]):
                        newlist.append(mybir.InstNoOp(
                            name=f"{inst.name}_waitnop{i}",
                            engine=inst.engine,
                            bass_nofuse=True,
                            sync_info=mybir.SyncInfo(on_wait=[w], on_update=[]),
                        ))
                    inst.sync_info = mybir.SyncInfo(
                        on_wait=waits[:cap], on_update=list(si.on_update))
                    changed = True
                newlist.append(inst)
            if changed:
                insts.clear()
                insts.extend(newlist)


# ---------------------------------------------------------------------------
# All DRAM tensors are staged host-side in partition-major, tile-contiguous
# layouts so every dma_start is a handful of large per-partition runs (fast
# HWDGE descriptor generation, full line rate):
#   x_d [128, NBT, 20, BT]   (p, batch-tile, block-k-half, batch)
#   w_d [128, 48, 256]       (p=k-half, j=(li,s,kh), q)
#   o_d [128, NBT, 8, 2, BT] (p, batch-tile, li, qh, batch)
# Startup interleaves per-li weight chunks with per-block x chunks on the
# single HW DMA queue so the first matmul only waits for ~1 MB, and each
# x tile is prefetched at the top of the previous tile's compute so it is
# not queued behind that tile's output stores.
def _build_nc(mode):
    dt_in, _ = _DT[mode]
    f32 = mybir.dt.float32
    # bf16 mode also writes the output in bf16 (host upcasts): halves the
    # store traffic, well inside the 2e-2 gate.
    dt_out = dt_in if mode == "bf16" else f32
    nc = bass.Bass()
    x_d = nc.dram_tensor("x", [128, NBT, NHALO * 2, BT], dt_in,
                         kind="ExternalInput")
    w_d = nc.dram_tensor("w", [128, BPC * 3 * 2, P], dt_in,
                         kind="ExternalInput")
    o_d = nc.dram_tensor("o", [128, NBT, BPC, 2, BT], dt_out,
                         kind="ExternalOutput")

    with TileContext(nc) as tc:
        with tc.tile_pool(name="wpool", bufs=1) as wpool, \
             tc.tile_pool(name="xpool", bufs=2) as xpool, \
             tc.tile_pool(name="opool", bufs=4) as opool, \
             tc.tile_pool(name="pspool", bufs=6, space="PSUM") as pspool, \
             tc.tile_pool(name="ps2pool", bufs=2, space="PSUM") as ps2pool:
            w_sb = wpool.tile([128, BPC * 3 * 2, P], dt_in)
            xt0 = xpool.tile([128, NHALO * 2, BT], dt_in, tag="x")

            # PE warm-up: the HAM clock gate holds the PE at 1.2 GHz until
            # ~3.4 us of sustained activity. Run dummy matmuls on zeroed
            # scratch during the startup-DMA dead time so the real stream
            # begins (nearly) at 2.4 GHz. Sized to finish just before the
            # first x block lands (~10 us).
            warm_w = wpool.tile([128, 128], dt_in, tag="warmw")
            warm_x = wpool.tile([128, BT], dt_in, tag="warmx")
            nc.vector.memset(warm_w, 0.0)
            nc.vector.memset(warm_x, 0.0)
            ps_warm = pspool.tile([128, BT], f32, tag="ps")
            for _ in range(7):
                nc.tensor.matmul(ps_warm, warm_w, warm_x,
                                 start=True, stop=True)

            def w_chunk(j0, j1):
                nc.sync.dma_start(out=w_sb[:, j0:j1, :], in_=w_d[:, j0:j1, :])

            def x0_chunk(blk):
                nc.sync.dma_start(out=xt0[:, blk * 2:blk * 2 + 2, :],
                                  in_=x_d[:, 0, blk * 2:blk * 2 + 2, :])

            # startup drip, sized so HWDGE descriptor generation (~0.5 us
            # per dma_start on the sync engine) stays ahead of the queue:
            # the first group needs w[0:6] + x blocks 0-2, each li after
            # consumes one x block / 2.56 us of compute.
            w_chunk(0, 2)
            x0_chunk(0)
            w_chunk(2, 6)
            x0_chunk(1)
            x0_chunk(2)
            for li in range(1, BPC):
                if li % 2 == 1:
                    w_chunk(li * 6, min((li + 2) * 6, BPC * 6))
                x0_chunk(li + 2)

            xt = xt0
            for bt in range(NBT):
                # stores ride the scalar engine's own HW queue
                # (qActDynamicHW) so they never head-of-line block x loads
                # on the sync queue, and vice versa.
                last_tile = bt == NBT - 1
                if not last_tile:
                    xt_next = xpool.tile([128, NHALO * 2, BT], dt_in, tag="x")
                    nc.sync.dma_start(out=xt_next, in_=x_d[:, bt + 1])
                for g in range(BPC // 2):   # output pairs of block rows
                    for lj in range(2):
                        li = g * 2 + lj
                        if last_tile or lj == 0:
                            ot = opool.tile(
                                [128, 2, 2, BT], dt_out, tag="o")
                        for qh in range(2):
                            final = last_tile and li == BPC - 1 and qh == 1
                            # the very last group runs as two 256-column
                            # half-groups so the cast+store chain after the
                            # final matmul is half as long
                            bhs = (0, 1) if final else (0,)
                            for bh in bhs:
                                bsl = (slice(bh * 256, bh * 256 + 256)
                                       if final else slice(None))
                                if final:
                                    ps = ps2pool.tile([128, 256], f32,
                                                      tag="p2", name="ps2")
                                else:
                                    ps = pspool.tile([128, BT], f32,
                                                     tag="ps", name="ps")
                                for s in range(3):
                                    for kh in range(2):
                                        nc.tensor.matmul(
                                            ps,
                                            w_sb[:, (li * 3 + s) * 2 + kh,
                                                 qh * 128:(qh + 1) * 128],
                                            xt[:, (li + s) * 2 + kh, bsl],
                                            start=(s == 0 and kh == 0),
                                            stop=(s == 2 and kh == 1),
                                        )
                                nc.vector.tensor_copy(
                                    out=ot[:, 0 if last_tile else lj, qh,
                                           bsl],
                                    in_=ps)
                                if final:
                                    # alternate engines so the two halves'
                                    # descriptor generation runs in parallel
                                    eng = nc.scalar if bh == 0 else nc.sync
                                    eng.dma_start(
                                        out=o_d[:, bt, li:li + 1, qh:qh + 1,
                                                bsl],
                                        in_=ot[:, 0:1, qh:qh + 1, bsl])
                            if last_tile and li == BPC - 1 and qh == 0:
                                nc.scalar.dma_start(
                                    out=o_d[:, bt, li:li + 1, 0:1, :],
                                    in_=ot[:, 0:1, 0:1, :])
                        if last_tile and li < BPC - 1:
                            # per-block stores drain the store queue before
                            # the final matmul, shrinking the exit tail
                            nc.scalar.dma_start(
                                out=o_d[:, bt, li:li + 1, :, :],
                                in_=ot[:, 0:1, :, :])
                    if not last_tile:
                        nc.scalar.dma_start(
                            out=o_d[:, bt, g * 2:g * 2 + 2, :, :], in_=ot)
                if not last_tile:
                    xt = xt_next
    _hoist_excess_waits(nc)
    return nc


def _host_prep(x, Wd, Wu, Wl, Wtr, Wbl, np_dt):
    x = np.asarray(x, np.float32)
    Wd, Wu, Wl = np.asarray(Wd, np.float32), np.asarray(Wu, np.float32), np.asarray(Wl, np.float32)
    Wtr, Wbl = np.asarray(Wtr, np.float32), np.asarray(Wbl, np.float32)

    xT = np.ascontiguousarray(x.T).astype(np_dt)         # [M*P, B]
    A = np.concatenate([Wtr[None], Wl], axis=0)          # weight applied to x_{i-1}
    Bst = Wd                                             # weight applied to x_i
    C = np.concatenate([Wu, Wbl[None]], axis=0)          # weight applied to x_{i+1}
    WT = np.stack([A, Bst, C], axis=1)                   # [64, 3, q, p]
    WT = np.ascontiguousarray(WT.transpose(0, 1, 3, 2))  # [64, 3, p, q]

    in_maps = []
    for c in range(NCORES):
        lo = (8 * c - 1) * P
        hi = (8 * c + 9) * P
        if lo < 0:
            xc = np.concatenate([xT[lo:], xT[:hi]], axis=0)
        elif hi > M * P:
            xc = np.concatenate([xT[lo:], xT[:hi - M * P]], axis=0)
        else:
            xc = xT[lo:hi]                               # [2560, 4096]
        # [2560, B] -> [128p, NBT, 20t, BT] partition-major tiled
        xc = np.ascontiguousarray(
            xc.reshape(NHALO * 2, 128, NBT, BT).transpose(1, 2, 0, 3))
        wc = WT[8 * c:8 * c + 8].reshape(BPC * 3 * 2, 128, P).astype(np_dt)
        wc = np.ascontiguousarray(wc.transpose(1, 0, 2))  # [128, 48, 256]
        in_maps.append({"x": xc, "w": wc})
    return in_maps


def kernel(x, Wd, Wu, Wl, Wtr, Wbl):
    global LAST_EXEC_NS
    _apply_tile_patch()
    if TRACE:
        try:
            _install_profile_shim()
        except Exception as e:
            print(f"profile shim failed ({e}); running without trace")
    dt_in, np_dt = _DT[MODE]
    nc = _build_nc(MODE)
    in_maps = _host_prep(x, Wd, Wu, Wl, Wtr, Wbl, np_dt)
    res = run_bass_kernel_spmd(
        nc, in_maps, core_ids=list(range(NCORES)), trace=TRACE)
    LAST_EXEC_NS = res.exec_time_ns
    if TRACE and REPEATS > 1:
        global ALL_EXEC_NS
        ALL_EXEC_NS = [res.exec_time_ns]
        for _ in range(REPEATS - 1):
            r2 = run_bass_kernel_spmd(
                nc, in_maps, core_ids=list(range(NCORES)), trace=True)
            ALL_EXEC_NS.append(r2.exec_time_ns)
        LAST_EXEC_NS = min(t for t in ALL_EXEC_NS if t)
    out = np.empty((B, M, P), dtype=np.float32)
    for c in range(NCORES):
        oc = res.results[c]["o"]                  # [128p, NBT, 8li, 2qh, BT]
        # -> [bt, b, li, qh, p] -> [B, 8, 256]
        out[:, 8 * c:8 * c + 8, :] = (
            oc.transpose(1, 4, 2, 3, 0).reshape(B, BPC, P))
    return np.ascontiguousarray(out.reshape(B, M * P))   # [B, M*P] float32

